# revision 21
# baseline (speedup 1.0000x reference)
"""Trainium2 Bass kernel for nn_AttentionSubModule: batched tiny attention.

Per item (131072 total): x row of 225 = 25 tokens x 9 dims, 4 token groups
each with own 9x9 Wq/Wk/Wv + bias; scores = qk^T/3 (+mask*-1e9), softmax,
out = attn@v + residual, LayerNorm over the 9-dim axis.

Mapping: pure data parallel over 8 cores (16384 items each), supertiles of
NB*128 items (items on partitions, NB blocks in the free dim).

Engine split (v2 cost model):
 - PE: x transpose, q/k/v projections as block-diag matmuls in transposed
   layout with bias folded in via a constant ones-row, transpose back.
 - ACT: PSUM evacuations (downcast to f16), exp, rsqrt via ln+exp (all
   funcs in the natural_log_exp_and_others table -> no table reloads).
 - DVE: the two big broadcast products (f16, 2x mode) + 2x-eligible
   reduce-tree stages + bn_stats for LayerNorm stats.
 - Pool(gpsimd): 1x tree bottoms and the LN tail via scalar_tensor_tensor
   (0.6 impl efficiency beats tensor_tensor's 0.42).

Algebra: softmax division folded away via LN scale invariance
(LN(attn@v/Z + x) == LN(attn_unnorm@v + Z*x)); mask exp-weights and the
1/sqrt(9) score scale folded into the host-side V/Q weights; Z obtained by
appending an expm row to the e-major V tile.
"""

import numpy as np
from contextlib import ExitStack

import concourse.bass as bass
import concourse.tile as tile
from concourse import mybir
from concourse.bass_utils import run_bass_kernel_spmd

KV = 9
NQ = 25
D = NQ * KV  # 225
GROUPS = [(0, 27, 3), (27, 117, 10), (117, 207, 10), (207, 225, 2)]
N_CORES = 8
P = 128
EPS = 1e-5
F32 = mybir.dt.float32
F16 = mybir.dt.float16

NA = 14 * KV   # chunk A: tokens 0..13 -> 126 rows
NB_ = 11 * KV  # chunk B: tokens 14..24 -> 99 rows
NBLK = 2       # blocks of 128 items per supertile

# pm16 (f16 per-partition consts) column layout:
#   [0:128)   identity f16 128x128
#   then MqA, MkA, MvA (126 cols each; rows 0:126 = W^T blockdiag, row 126 = bias)
#   then MqB, MkB, MvB (99 cols each; rows 0:99, row 99 = bias)
PM16_COLS = 128 + 3 * NA + 3 * NB_

# cst (broadcast f32 consts): [gamma 9 | beta 9 | expm 25 | shift 1]
CST_LEN = KV + KV + NQ + 1

AF = mybir.ActivationFunctionType
ALU = mybir.AluOpType
AX = mybir.AxisListType
SHIFT = -8.0  # exp(s - 8) keeps f16 attn weights in range; absorbed by LN


def _bcast_ap(handle, n_part):
    ap = handle[:]
    return bass.AP(tensor=ap.tensor, offset=ap.offset, ap=[[0, n_part]] + list(ap.ap))


def build_program(b_core, probe=9):
    assert b_core % (P * NBLK) == 0
    nsuper = b_core // (P * NBLK)
    nc = bass.Bass("TRN2", target_bir_lowering=False)

    x_d = nc.dram_tensor("x", [b_core, D], F32, kind="ExternalInput")
    cst_d = nc.dram_tensor("cst", [CST_LEN], F32, kind="ExternalInput")
    id32_d = nc.dram_tensor("id32", [P, P], F32, kind="ExternalInput")
    pm16_d = nc.dram_tensor("pm16", [P, PM16_COLS], F16, kind="ExternalInput")
    ones16_d = nc.dram_tensor("ones16", [NBLK * P], F16, kind="ExternalInput")
    out_d = nc.dram_tensor("out", [b_core, D], F32, kind="ExternalOutput")

    with tile.TileContext(nc) as tc, ExitStack() as ctx:
        consts = ctx.enter_context(tc.tile_pool(name="consts", bufs=1))
        xin = ctx.enter_context(tc.tile_pool(name="xin", bufs=3))
        tlay = ctx.enter_context(tc.tile_pool(name="tlay", bufs=2))
        proj = ctx.enter_context(tc.tile_pool(name="proj", bufs=3))
        big = ctx.enter_context(tc.tile_pool(name="big", bufs=2))
        sm = ctx.enter_context(tc.tile_pool(name="sm", bufs=2))
        outp = ctx.enter_context(tc.tile_pool(name="outp", bufs=2))
        psum = ctx.enter_context(tc.tile_pool(name="psum", bufs=1, space="PSUM"))

        # ---- constants ----
        cst_t = consts.tile([P, CST_LEN], F32)
        nc.gpsimd.dma_start(out=cst_t, in_=_bcast_ap(cst_d, P))
        g_t = cst_t[:, 0:KV]
        b_t = cst_t[:, KV : 2 * KV]
        expm_t = cst_t[:, 2 * KV : 2 * KV + NQ]
        shift_t = cst_t[:, CST_LEN - 1 : CST_LEN]

        id32_t = consts.tile([P, P], F32)
        nc.sync.dma_start(out=id32_t, in_=id32_d[:, :])
        ident32 = id32_t[:, 0:P]

        pm16_t = consts.tile([P, PM16_COLS], F16)
        nc.sync.dma_start(out=pm16_t, in_=pm16_d[:, :])
        ident16 = pm16_t[:, 0:P]
        o = P
        stA = {}
        stB = {}
        for nm in ("q", "k", "v"):
            stA[nm] = pm16_t[0 : NA + 1, o : o + NA]
            o += NA
        for nm in ("q", "k", "v"):
            stB[nm] = pm16_t[0 : NB_ + 1, o : o + NB_]
            o += NB_
        assert o == PM16_COLS

        inv_sqrt_kv = float(1.0 / np.sqrt(KV))

        # ---- persistent-buffer prologue: ones rows in xT, expm row in vE ----
        # Pools rotate buffers per tag; pull each buffer once and prefill the
        # rows that the per-iteration writes never touch.
        xT1s, xT2s, vEs = [], [], []
        for _ in range(2):
            t1 = tlay.tile([NA + 1, NBLK, P], F16, tag="xT1")
            nc.sync.dma_start(
                out=t1[NA : NA + 1, :, :],
                in_=ones16_d[:].rearrange("(o b p) -> o b p", o=1, b=NBLK),
            )
            xT1s.append(t1)
            t2 = tlay.tile([NB_ + 1, NBLK, P], F16, tag="xT2")
            nc.sync.dma_start(
                out=t2[NB_ : NB_ + 1, :, :],
                in_=ones16_d[:].rearrange("(o b p) -> o b p", o=1, b=NBLK),
            )
            xT2s.append(t2)
        for _ in range(3):
            ve = proj.tile([P, NBLK, KV + 1, NQ], F16, tag="ve")
            for b in range(NBLK):
                nc.gpsimd.tensor_copy(ve[:, b, KV, :], expm_t)
            vEs.append(ve)

        states = []

        def phase1(t):
            r0 = t * P * NBLK
            xsrc = x_d[r0 : r0 + P * NBLK, :].rearrange("(b p) c -> p b c", b=NBLK)
            xt = xin.tile([P, NBLK, D], F32, tag="x")
            nc.sync.dma_start(out=xt, in_=xsrc)

            if probe == 0:
                o_t0 = outp.tile([P, NBLK, D], F32, tag="o")
                nc.vector.tensor_copy(o_t0[:], xt[:])
                nc.sync.dma_start(
                    out=out_d[r0 : r0 + P * NBLK, :].rearrange(
                        "(b p) c -> p b c", b=NBLK
                    ),
                    in_=o_t0[:],
                )
                return

            # ---- transpose x to feature-major (per 128-block) ----
            psx = psum.tile([NA, NBLK, 2, P], F32, tag="psx")
            for b in range(NBLK):
                nc.tensor.transpose(psx[:, b, 0, :], xt[:, b, 0:NA], ident32)
                nc.tensor.transpose(psx[0:NB_, b, 1, :], xt[:, b, NA:D], ident32)
            xT1 = xT1s[t % 2]
            xT2 = xT2s[t % 2]
            nc.scalar.copy(xT1[0:NA, :, :], psx[:, :, 0, :])
            nc.scalar.copy(xT2[0:NB_, :, :], psx[0:NB_, :, 1, :])

            # ---- q/k/v projections (bias via ones-row) ----
            qkv_ps = psum.tile([NA, 6, NBLK * P], F32, tag="qkv")
            rhsA = xT1[:].rearrange("r b p -> r (b p)")
            rhsB = xT2[:].rearrange("r b p -> r (b p)")
            for j, nm in enumerate(("q", "k", "v")):
                nc.tensor.matmul(qkv_ps[:, j, :], stA[nm], rhsA, start=True, stop=True)
                nc.tensor.matmul(
                    qkv_ps[0:NB_, 3 + j, :], stB[nm], rhsB, start=True, stop=True
                )
            sTA = tlay.tile([NA, 3, NBLK * P], F16, tag="sTA")
            sTB = tlay.tile([NB_, 3, NBLK * P], F16, tag="sTB")
            nc.scalar.copy(sTA[:], qkv_ps[:, 0:3, :])
            nc.scalar.copy(sTB[:], qkv_ps[0:NB_, 3:6, :])

            # ---- transpose back to item-rows; pack q,k then v per block ----
            qk = proj.tile([P, NBLK, 2, NQ, KV], F16, tag="qk")
            vE = vEs[t % 3]
            for b in range(NBLK):
                # 226-wide rows keep every f16 PSUM write 4-byte aligned
                qvT = psum.tile([P, 3, D + 1], F16, tag=f"qvT{b}")
                for j in range(3):
                    nc.tensor.transpose(
                        qvT[:, j, 0:NA],
                        sTA[:, j, b * P : (b + 1) * P],
                        ident16[0:NA, 0:NA],
                    )
                    nc.tensor.transpose(
                        qvT[:, j, NA:D],
                        sTB[:, j, b * P : (b + 1) * P],
                        ident16[0:NB_, 0:NB_],
                    )
                nc.scalar.copy(
                    qk[:, b, :, :, :].rearrange("p a i d -> p (a i d)").rearrange(
                        "p (a c) -> p a c", a=2
                    ),
                    qvT[:, 0:2, 0:D],
                )
                # scatter v into e-major rows 0..8 of vE (row 9 = expm, prefilled)
                nc.scalar.copy(
                    vE[:, b, 0:KV, :].transpose([0, 2, 1]),
                    qvT[:, 2, 0:D].rearrange("p (i d) -> p i d", i=NQ),
                )

            q_t = qk[:, :, 0, :, :]
            k_t = qk[:, :, 1, :, :]

            if probe == 1:
                o_t1 = outp.tile([P, NBLK, D], F32, tag="o")
                nc.vector.tensor_add(
                    o_t1[:].rearrange("p b (i d) -> p b i d", i=NQ), q_t, k_t
                )
                nc.sync.dma_start(
                    out=out_d[r0 : r0 + P * NBLK, :].rearrange(
                        "(b p) c -> p b c", b=NBLK
                    ),
                    in_=o_t1[:],
                )
                return

            # ---- scores: products (DVE 2x) + tree (top DVE, bottom Pool) ----
            pr2 = big.tile([P, NBLK, NQ, NQ, KV], F16, tag="pr2")
            for b in range(NBLK):
                nc.vector.tensor_mul(
                    pr2[:, b],
                    q_t[:, b].unsqueeze(2).broadcast_to((P, NQ, NQ, KV)),
                    k_t[:, b].unsqueeze(1).broadcast_to((P, NQ, NQ, KV)),
                )
            t1sc = sm.tile([P, NBLK, NQ * NQ, 4], F16, tag="t1sc", bufs=1)
            nc.vector.tensor_add(
                t1sc[:],
                pr2[:].rearrange("p b i j d -> p b (i j) d")[:, :, :, 0:4],
                pr2[:].rearrange("p b i j d -> p b (i j) d")[:, :, :, 4:8],
            )
            t2sc = sm.tile([P, NBLK, NQ * NQ, 2], F16, tag="t2sc")
            nc.vector.tensor_add(t2sc[:], t1sc[:, :, :, 0:2], t1sc[:, :, :, 2:4])
            t3sc = sm.tile([P, NBLK, NQ * NQ], F16, tag="t3sc")
            sc = sm.tile([P, NBLK, NQ * NQ], F32, tag="sc")
            ex = sm.tile([P, NBLK, NQ, NQ], F16, tag="ex")
            pr2f = pr2[:].rearrange("p b i j d -> p b (i j) d")
            for b in range(NBLK):
                nc.vector.tensor_add(t3sc[:, b], t2sc[:, b, :, 0], t2sc[:, b, :, 1])
                nc.vector.tensor_add(sc[:, b], t3sc[:, b], pr2f[:, b, :, 8])
                # exp (ACT); shift keeps f16 range, absorbed by LN
                nc.scalar.activation(
                    ex[:, b].rearrange("p i j -> p (i j)"), sc[:, b],
                    AF.Exp, bias=shift_t, scale=inv_sqrt_kv,
                )

            if probe == 2:
                o_t2 = outp.tile([P, NBLK, D], F32, tag="o")
                nc.vector.tensor_copy(
                    o_t2[:].rearrange("p b (i d) -> p b i d", i=NQ)[:, :, :, 0:KV],
                    ex[:, :, :, 0:KV],
                )
                nc.sync.dma_start(
                    out=out_d[r0 : r0 + P * NBLK, :].rearrange(
                        "(b p) c -> p b c", b=NBLK
                    ),
                    in_=o_t2[:],
                )
                return

            # ---- attn @ v (+ Z via expm row): products + tree over j ----
            # pr3[p,b,i,e,j] = ex[p,b,i,j] * vE[p,b,e,j]; e=9 gives Z terms
            pr3 = big.tile([P, NBLK, NQ, KV + 1, NQ], F16, tag="pr3")
            for b in range(NBLK):
                nc.vector.tensor_mul(
                    pr3[:, b],
                    ex[:, b].unsqueeze(2).broadcast_to((P, NQ, KV + 1, NQ)),
                    vE[:, b].unsqueeze(1).broadcast_to((P, NQ, KV + 1, NQ)),
                )
            # av tree scratch aliases pr2's storage (dead after sc/r4 above)
            G = NQ * (KV + 1)  # 250 groups per block
            pr2flat = pr2[:].rearrange("p b i j d -> p b (i j d)")
            p3 = pr3[:].rearrange("p b i e j -> p b (i e) j")
            t1av = pr2flat[:, :, 0:G * 12].rearrange("p b (g w) -> p b g w", w=12)
            nc.vector.tensor_add(t1av, p3[:, :, :, 0:12], p3[:, :, :, 12:24])
            t2av = pr2flat[:, :, G * 12 : G * 18].rearrange(
                "p b (g w) -> p b g w", w=6
            )
            nc.vector.tensor_add(t2av, t1av[:, :, :, 0:6], t1av[:, :, :, 6:12])
            t3av_t = sm.tile([P, NBLK, G, 3], F16, tag="t3av")
            t3av = t3av_t[:]
            nc.vector.tensor_add(t3av, t2av[:, :, :, 0:3], t2av[:, :, :, 3:6])
            states.append(dict(t=t, r0=r0, xt=xt, t3av=t3av, p3=p3))


        def phase2(st):
            t, r0, xt, t3av, p3 = st["t"], st["r0"], st["xt"], st["t3av"], st["p3"]
            G = NQ * (KV + 1)
            s01 = sm.tile([P, NBLK, G], F16, tag="s01")
            nc.gpsimd.tensor_add(s01[:], t3av[:, :, :, 0], t3av[:, :, :, 1])
            s012 = sm.tile([P, NBLK, G], F16, tag="s012")
            nc.gpsimd.tensor_add(s012[:], s01[:], t3av[:, :, :, 2])
            u_t = outp.tile([P, NBLK, NQ, KV + 1], F32, tag="u")
            nc.gpsimd.tensor_add(
                u_t[:].rearrange("p b i e -> p b (i e)"), s012[:], p3[:, :, :, 24]
            )

            z_t = u_t[:, :, :, KV]  # [P, NBLK, NQ]

            if probe == 3:
                o_t3 = outp.tile([P, NBLK, D], F32, tag="o")
                nc.vector.tensor_mul(
                    o_t3[:].rearrange("p b (i d) -> p b i d", i=NQ),
                    xt[:].rearrange("p b (i d) -> p b i d", i=NQ),
                    z_t.unsqueeze(3).broadcast_to((P, NBLK, NQ, KV)),
                )
                nc.sync.dma_start(
                    out=out_d[r0 : r0 + P * NBLK, :].rearrange(
                        "(b p) c -> p b c", b=NBLK
                    ),
                    in_=o_t3[:],
                )
                return

            # ---- w = u + Z*x  (LN scale-invariance absorbs Z) ----
            # 3D views: [P, (b i), d] -- HW tensor ops allow <=3 free dims;
            # Pool engine only implements TensorTensor add/sub/mult (+copy).
            xv3 = xt[:].rearrange("p b (i d) -> p (b i) d", i=NQ)
            zb3 = z_t.rearrange("p b i -> p (b i)").unsqueeze(2).broadcast_to(
                (P, NBLK * NQ, KV)
            )
            zx = outp.tile([P, NBLK, NQ, KV], F32, tag="zx")
            zx3 = zx[:].rearrange("p b i d -> p (b i) d")
            nc.gpsimd.tensor_mul(zx3, xv3, zb3)
            w_t = outp.tile([P, NBLK, NQ, KV], F32, tag="w")
            w3 = w_t[:].rearrange("p b i d -> p (b i) d")
            nc.gpsimd.tensor_add(
                w3, u_t[:, :, :, 0:KV].rearrange("p b i d -> p (b i) d"), zx3
            )

            # wsq = (3w)^2 = 9w^2 (scale folded into ACT square)
            wsq = outp.tile([P, NBLK, NQ, KV], F32, tag="wsq")
            nc.scalar.activation(
                wsq[:].rearrange("p b i d -> p (b i d)"),
                w_t[:].rearrange("p b i d -> p (b i d)"),
                AF.Square, bias=0.0, scale=3.0,
            )
            # Pool 3+3+3 trees: sw = sum_e w ; sq9 = sum_e 9w^2
            sw = sm.tile([P, NBLK * NQ], F32, tag="sw")
            sq9 = sm.tile([P, NBLK * NQ], F32, tag="sq9")
            for srct, dst, tg in ((w_t, sw, "rw"), (wsq, sq9, "rq")):
                s3 = srct[:].rearrange("p b i d -> p (b i) d")
                r1 = sm.tile([P, NBLK * NQ, 3], F32, tag=f"{tg}1")
                nc.gpsimd.tensor_add(r1[:], s3[:, :, 0:3], s3[:, :, 3:6])
                r2 = sm.tile([P, NBLK * NQ, 3], F32, tag=f"{tg}2")
                nc.gpsimd.tensor_add(r2[:], r1[:], s3[:, :, 6:9])
                r3 = sm.tile([P, NBLK * NQ], F32, tag=f"{tg}3")
                nc.gpsimd.tensor_add(r3[:], r2[:, :, 0], r2[:, :, 1])
                nc.gpsimd.tensor_add(dst[:], r3[:], r2[:, :, 2])

            # var*81 = 9*sum(9w^2)/9... : v2 = sq9*... -- algebra:
            # sum 9w^2 = 9*sum w^2 ; var*81 = 9*(9*sum w^2) - (3*sw)^2... use:
            # 81*var = 9*sq9 - 9*sw^2 -> instead: v2 = sq9 - s2 where
            # s2 = sw^2 gives 9*(sum w^2) - sw^2 = 9*var*9 = 81*var/... :
            # sum w^2 = sq9/9; 81*var = 9*(9 sum w^2 - sw^2) = 9*(sq9 - sw^2)
            # so compute v2 = sq9 - s2, then fold the final *9 into z2/ln:
            # 81*(var + eps*Z^2) = 9*v2 + 81*eps*Z^2 = 9*(v2 + 9*eps*Z^2)
            s2 = sm.tile([P, NBLK * NQ], F32, tag="s2")
            nc.gpsimd.tensor_mul(s2[:], sw[:], sw[:])
            v2 = sm.tile([P, NBLK * NQ], F32, tag="v2")
            nc.gpsimd.tensor_sub(v2[:], sq9[:], s2[:])
            # z2 = (9*sqrt(eps)*Z)^2 = 81*eps*Z^2  (scale folded into square)
            z2 = sm.tile([P, NBLK * NQ], F32, tag="z2")
            nc.scalar.activation(
                z2[:], z_t.rearrange("p b i -> p (b i)"),
                AF.Square, bias=0.0, scale=float(9.0 * np.sqrt(EPS)),
            )
            v3 = sm.tile([P, NBLK * NQ], F32, tag="v3")
            nc.gpsimd.tensor_add(v3[:], z2[:], v2[:])
            # v3 = 81*(var + eps*Z^2) so exp(-ln(v3)/2) = rstd_true/9
            # -> gamma*9 on host
            lnv = sm.tile([P, NBLK * NQ], F32, tag="lnv")
            nc.scalar.activation(lnv[:], v3[:], AF.Ln, bias=0.0, scale=1.0)
            rstd = sm.tile([P, NBLK * NQ], F32, tag="rstd")
            nc.scalar.activation(rstd[:], lnv[:], AF.Exp, bias=0.0, scale=-0.5)

            # cen = w - sw/9  (the -1/9 scale via ACT copy)
            swm = sm.tile([P, NBLK * NQ], F32, tag="swm")
            nc.scalar.activation(
                swm[:], sw[:], AF.Copy, bias=0.0, scale=float(-1.0 / 9.0)
            )
            cen = outp.tile([P, NBLK, NQ, KV], F32, tag="cen")
            cen3 = cen[:].rearrange("p b i d -> p (b i) d")
            nc.gpsimd.tensor_add(
                cen3, w3, swm[:].unsqueeze(2).broadcast_to((P, NBLK * NQ, KV))
            )
            t4 = outp.tile([P, NBLK, NQ, KV], F32, tag="t4")
            t43 = t4[:].rearrange("p b i d -> p (b i) d")
            nc.gpsimd.tensor_mul(
                t43, cen3, rstd[:].unsqueeze(2).broadcast_to((P, NBLK * NQ, KV))
            )
            og = outp.tile([P, NBLK, NQ, KV], F32, tag="og")
            og3 = og[:].rearrange("p b i d -> p (b i) d")
            gb3 = g_t.unsqueeze(1).broadcast_to((P, NBLK * NQ, KV))
            bb3 = b_t.unsqueeze(1).broadcast_to((P, NBLK * NQ, KV))
            nc.gpsimd.tensor_mul(og3, t43, gb3)
            o_t = outp.tile([P, NBLK, NQ, KV], F32, tag="o")
            nc.gpsimd.tensor_add(o_t[:].rearrange("p b i d -> p (b i) d"), og3, bb3)
            nc.sync.dma_start(
                out=out_d[r0 : r0 + P * NBLK, :].rearrange("(b p) c -> p b c", b=NBLK),
                in_=o_t[:].rearrange("p b i d -> p b (i d)"),
            )


        for t in range(nsuper):
            phase1(t)
            if probe >= 4 and states:
                phase2(states.pop(0))
        while states:
            phase2(states.pop(0))

    _split_multi_waits(nc)
    return nc


def _split_multi_waits(nc):
    """Walrus allows only one sync-wait slot on most instruction encodings.
    Hoist excess waits into NoOps inserted just before the offender."""
    for f in nc.m.functions:
        for b in f.blocks:
            i = 0
            while i < len(b.instructions):
                inst = b.instructions[i]
                si = getattr(inst, "sync_info", None)
                if si is not None and si.on_wait and len(si.on_wait) > 1:
                    extra = si.on_wait[:-1]
                    si.on_wait = si.on_wait[-1:]
                    for w in extra:
                        nop = mybir.InstNoOp(
                            name=nc.get_next_instruction_name(),
                            engine=inst.engine,
                            ins=[],
                            outs=[],
                            sync_info=mybir.SyncInfo(on_wait=[w], on_update=[]),
                        )
                        nc.register_instruction(nop)
                        b.instructions.insert(i, nop)
                        i += 1
                i += 1
    return nc


_NC_CACHE = {}


def _get_program(b_core):
    if b_core not in _NC_CACHE:
        _NC_CACHE[b_core] = build_program(b_core)
    return _NC_CACHE[b_core]


def _host_consts(mask, Wq, bq, Wk, bk, Wv, bv, gamma, beta):
    """Build pm16 [128, PM16_COLS] f16 and cst [CST_LEN] f32."""
    gidx = np.empty(NQ, dtype=np.int64)
    for g, (s, e, n) in enumerate(GROUPS):
        gidx[s // KV : e // KV] = g

    mask = np.asarray(mask, np.float64)
    expm = np.exp(np.float64(-1e9) * mask).astype(np.float32)  # mask weights

    def mk_stationary(W, b_, tok_lo, tok_hi, scale_tok=None):
        n = (tok_hi - tok_lo) * KV
        M = np.zeros((n + 1, n), dtype=np.float32)
        for i in range(tok_lo, tok_hi):
            s = 1.0 if scale_tok is None else scale_tok[i]
            r = (i - tok_lo) * KV
            M[r : r + KV, r : r + KV] = np.asarray(W, np.float32)[gidx[i]].T * s
            M[n, r : r + KV] = np.asarray(b_, np.float32)[gidx[i]] * s
        return M

    pm16 = np.zeros((P, PM16_COLS), dtype=np.float16)
    pm16[:, 0:P] = np.eye(P, dtype=np.float16)
    o = P
    for W, b_, sc in ((Wq, bq, None), (Wk, bk, None), (Wv, bv, expm)):
        M = mk_stationary(W, b_, 0, 14, sc)
        pm16[0 : NA + 1, o : o + NA] = M.astype(np.float16)
        o += NA
    for W, b_, sc in ((Wq, bq, None), (Wk, bk, None), (Wv, bv, expm)):
        M = mk_stationary(W, b_, 14, 25, sc)
        pm16[0 : NB_ + 1, o : o + NB_] = M.astype(np.float16)
        o += NB_
    assert o == PM16_COLS

    cst = np.concatenate([
        9.0 * np.asarray(gamma, np.float32).reshape(-1),  # rstd is rstd_true/9
        np.asarray(beta, np.float32).reshape(-1),
        expm.reshape(-1),
        np.array([SHIFT], np.float32),
    ]).astype(np.float32)
    assert cst.shape[0] == CST_LEN
    id32 = np.eye(P, dtype=np.float32)
    return pm16, cst, id32


def kernel(x, mask, Wq, bq, Wk, bk, Wv, bv, gamma, beta):
    x = np.ascontiguousarray(np.asarray(x, dtype=np.float32))
    B = x.shape[0]
    b_core = B // N_CORES
    pm16, cst, id32 = _host_consts(mask, Wq, bq, Wk, bk, Wv, bv, gamma, beta)

    nc = _get_program(b_core)
    shards = x.reshape(N_CORES, b_core, D)
    in_maps = []
    for c in range(N_CORES):
        in_maps.append({
            "x": np.ascontiguousarray(shards[c]),
            "cst": cst,
            "id32": id32,
            "pm16": pm16,
            "ones16": np.ones((NBLK * P,), dtype=np.float16),
        })
    res = run_bass_kernel_spmd(nc, in_maps, core_ids=list(range(N_CORES)))
    outs = [res.results[c]["out"] for c in range(N_CORES)]
    full = np.concatenate(outs, axis=0).reshape(B, NQ, KV)
    return full.astype(np.float32)


# revision 22
# speedup vs baseline: 1.0011x; 1.0011x over previous
"""Trainium2 Bass kernel for nn_AttentionSubModule: batched tiny attention.

Per item (131072 total): x row of 225 = 25 tokens x 9 dims, 4 token groups
each with own 9x9 Wq/Wk/Wv + bias; scores = qk^T/3 (+mask*-1e9), softmax,
out = attn@v + residual, LayerNorm over the 9-dim axis.

Mapping: pure data parallel over 8 cores (16384 items each), supertiles of
NB*128 items (items on partitions, NB blocks in the free dim).

Engine split (v2 cost model):
 - PE: x transpose, q/k/v projections as block-diag matmuls in transposed
   layout with bias folded in via a constant ones-row, transpose back.
 - ACT: PSUM evacuations (downcast to f16), exp, rsqrt via ln+exp (all
   funcs in the natural_log_exp_and_others table -> no table reloads).
 - DVE: the two big broadcast products (f16, 2x mode) + 2x-eligible
   reduce-tree stages + bn_stats for LayerNorm stats.
 - Pool(gpsimd): 1x tree bottoms and the LN tail via scalar_tensor_tensor
   (0.6 impl efficiency beats tensor_tensor's 0.42).

Algebra: softmax division folded away via LN scale invariance
(LN(attn@v/Z + x) == LN(attn_unnorm@v + Z*x)); mask exp-weights and the
1/sqrt(9) score scale folded into the host-side V/Q weights; Z obtained by
appending an expm row to the e-major V tile.
"""

import numpy as np
from contextlib import ExitStack

import concourse.bass as bass
import concourse.tile as tile
from concourse import mybir
from concourse.bass_utils import run_bass_kernel_spmd

KV = 9
NQ = 25
D = NQ * KV  # 225
GROUPS = [(0, 27, 3), (27, 117, 10), (117, 207, 10), (207, 225, 2)]
N_CORES = 8
P = 128
EPS = 1e-5
F32 = mybir.dt.float32
F16 = mybir.dt.float16

NA = 14 * KV   # chunk A: tokens 0..13 -> 126 rows
NB_ = 11 * KV  # chunk B: tokens 14..24 -> 99 rows
NBLK = 2       # blocks of 128 items per supertile

# pm16 (f16 per-partition consts) column layout:
#   [0:128)   identity f16 128x128
#   then MqA, MkA, MvA (126 cols each; rows 0:126 = W^T blockdiag, row 126 = bias)
#   then MqB, MkB, MvB (99 cols each; rows 0:99, row 99 = bias)
PM16_COLS = 128 + 3 * NA + 3 * NB_

# cst (broadcast f32 consts): [gamma 9 | beta 9 | expm 25 | shift 1]
CST_LEN = KV + KV + NQ + 1

AF = mybir.ActivationFunctionType
ALU = mybir.AluOpType
AX = mybir.AxisListType
SHIFT = -8.0  # exp(s - 8) keeps f16 attn weights in range; absorbed by LN


def _bcast_ap(handle, n_part):
    ap = handle[:]
    return bass.AP(tensor=ap.tensor, offset=ap.offset, ap=[[0, n_part]] + list(ap.ap))


def build_program(b_core, probe=9):
    assert b_core % (P * NBLK) == 0
    nsuper = b_core // (P * NBLK)
    nc = bass.Bass("TRN2", target_bir_lowering=False)

    x_d = nc.dram_tensor("x", [b_core, D], F32, kind="ExternalInput")
    cst_d = nc.dram_tensor("cst", [CST_LEN], F32, kind="ExternalInput")
    id32_d = nc.dram_tensor("id32", [P, P], F32, kind="ExternalInput")
    pm16_d = nc.dram_tensor("pm16", [P, PM16_COLS], F16, kind="ExternalInput")
    ones16_d = nc.dram_tensor("ones16", [NBLK * P], F16, kind="ExternalInput")
    out_d = nc.dram_tensor("out", [b_core, D], F32, kind="ExternalOutput")

    with tile.TileContext(nc) as tc, ExitStack() as ctx:
        consts = ctx.enter_context(tc.tile_pool(name="consts", bufs=1))
        xin = ctx.enter_context(tc.tile_pool(name="xin", bufs=3))
        tlay = ctx.enter_context(tc.tile_pool(name="tlay", bufs=2))
        proj = ctx.enter_context(tc.tile_pool(name="proj", bufs=3))
        big = ctx.enter_context(tc.tile_pool(name="big", bufs=2))
        sm = ctx.enter_context(tc.tile_pool(name="sm", bufs=2))
        outp = ctx.enter_context(tc.tile_pool(name="outp", bufs=2))
        psum = ctx.enter_context(tc.tile_pool(name="psum", bufs=1, space="PSUM"))

        # ---- constants ----
        cst_t = consts.tile([P, CST_LEN], F32)
        nc.gpsimd.dma_start(out=cst_t, in_=_bcast_ap(cst_d, P))
        g_t = cst_t[:, 0:KV]
        b_t = cst_t[:, KV : 2 * KV]
        expm_t = cst_t[:, 2 * KV : 2 * KV + NQ]
        shift_t = cst_t[:, CST_LEN - 1 : CST_LEN]

        id32_t = consts.tile([P, P], F32)
        nc.sync.dma_start(out=id32_t, in_=id32_d[:, :])
        ident32 = id32_t[:, 0:P]

        pm16_t = consts.tile([P, PM16_COLS], F16)
        nc.sync.dma_start(out=pm16_t, in_=pm16_d[:, :])
        ident16 = pm16_t[:, 0:P]
        o = P
        stA = {}
        stB = {}
        for nm in ("q", "k", "v"):
            stA[nm] = pm16_t[0 : NA + 1, o : o + NA]
            o += NA
        for nm in ("q", "k", "v"):
            stB[nm] = pm16_t[0 : NB_ + 1, o : o + NB_]
            o += NB_
        assert o == PM16_COLS

        inv_sqrt_kv = float(1.0 / np.sqrt(KV))

        # ---- persistent-buffer prologue: ones rows in xT, expm row in vE ----
        # Pools rotate buffers per tag; pull each buffer once and prefill the
        # rows that the per-iteration writes never touch.
        xT1s, xT2s, vEs = [], [], []
        for _ in range(2):
            t1 = tlay.tile([NA + 1, NBLK, P], F16, tag="xT1")
            nc.sync.dma_start(
                out=t1[NA : NA + 1, :, :],
                in_=ones16_d[:].rearrange("(o b p) -> o b p", o=1, b=NBLK),
            )
            xT1s.append(t1)
            t2 = tlay.tile([NB_ + 1, NBLK, P], F16, tag="xT2")
            nc.sync.dma_start(
                out=t2[NB_ : NB_ + 1, :, :],
                in_=ones16_d[:].rearrange("(o b p) -> o b p", o=1, b=NBLK),
            )
            xT2s.append(t2)
        for _ in range(3):
            ve = proj.tile([P, NBLK, KV + 1, NQ], F16, tag="ve")
            for b in range(NBLK):
                nc.gpsimd.tensor_copy(ve[:, b, KV, :], expm_t)
            vEs.append(ve)

        states = []

        def phase1(t):
            r0 = t * P * NBLK
            xsrc = x_d[r0 : r0 + P * NBLK, :].rearrange("(b p) c -> p b c", b=NBLK)
            xt = xin.tile([P, NBLK, D], F32, tag="x")
            nc.sync.dma_start(out=xt, in_=xsrc)

            if probe == 0:
                o_t0 = outp.tile([P, NBLK, D], F32, tag="o")
                nc.vector.tensor_copy(o_t0[:], xt[:])
                nc.sync.dma_start(
                    out=out_d[r0 : r0 + P * NBLK, :].rearrange(
                        "(b p) c -> p b c", b=NBLK
                    ),
                    in_=o_t0[:],
                )
                return

            # ---- transpose x to feature-major (per 128-block) ----
            psx = psum.tile([NA, NBLK, 2, P], F32, tag="psx")
            for b in range(NBLK):
                nc.tensor.transpose(psx[:, b, 0, :], xt[:, b, 0:NA], ident32)
                nc.tensor.transpose(psx[0:NB_, b, 1, :], xt[:, b, NA:D], ident32)
            xT1 = xT1s[t % 2]
            xT2 = xT2s[t % 2]
            nc.scalar.copy(xT1[0:NA, :, :], psx[:, :, 0, :])
            nc.scalar.copy(xT2[0:NB_, :, :], psx[0:NB_, :, 1, :])

            # ---- q/k/v projections (bias via ones-row) ----
            qkv_ps = psum.tile([NA, 6, NBLK * P], F32, tag="qkv")
            rhsA = xT1[:].rearrange("r b p -> r (b p)")
            rhsB = xT2[:].rearrange("r b p -> r (b p)")
            for j, nm in enumerate(("q", "k", "v")):
                nc.tensor.matmul(qkv_ps[:, j, :], stA[nm], rhsA, start=True, stop=True)
                nc.tensor.matmul(
                    qkv_ps[0:NB_, 3 + j, :], stB[nm], rhsB, start=True, stop=True
                )
            sTA = tlay.tile([NA, 3, NBLK * P], F16, tag="sTA")
            sTB = tlay.tile([NB_, 3, NBLK * P], F16, tag="sTB")
            nc.scalar.copy(sTA[:], qkv_ps[:, 0:3, :])
            nc.scalar.copy(sTB[:], qkv_ps[0:NB_, 3:6, :])

            # ---- transpose back to item-rows; pack q,k then v per block ----
            qk = proj.tile([P, NBLK, 2, NQ, KV], F16, tag="qk")
            vE = vEs[t % 3]
            for b in range(NBLK):
                # 226-wide rows keep every f16 PSUM write 4-byte aligned
                qvT = psum.tile([P, 3, D + 1], F16, tag=f"qvT{b}")
                for j in range(3):
                    nc.tensor.transpose(
                        qvT[:, j, 0:NA],
                        sTA[:, j, b * P : (b + 1) * P],
                        ident16[0:NA, 0:NA],
                    )
                    nc.tensor.transpose(
                        qvT[:, j, NA:D],
                        sTB[:, j, b * P : (b + 1) * P],
                        ident16[0:NB_, 0:NB_],
                    )
                nc.scalar.copy(
                    qk[:, b, :, :, :].rearrange("p a i d -> p (a i d)").rearrange(
                        "p (a c) -> p a c", a=2
                    ),
                    qvT[:, 0:2, 0:D],
                )
                # scatter v into e-major rows 0..8 of vE (row 9 = expm, prefilled)
                nc.scalar.copy(
                    vE[:, b, 0:KV, :].transpose([0, 2, 1]),
                    qvT[:, 2, 0:D].rearrange("p (i d) -> p i d", i=NQ),
                )

            q_t = qk[:, :, 0, :, :]
            k_t = qk[:, :, 1, :, :]

            if probe == 1:
                o_t1 = outp.tile([P, NBLK, D], F32, tag="o")
                nc.vector.tensor_add(
                    o_t1[:].rearrange("p b (i d) -> p b i d", i=NQ), q_t, k_t
                )
                nc.sync.dma_start(
                    out=out_d[r0 : r0 + P * NBLK, :].rearrange(
                        "(b p) c -> p b c", b=NBLK
                    ),
                    in_=o_t1[:],
                )
                return

            # ---- scores: products (DVE 2x) + tree (top DVE, bottom Pool) ----
            pr2 = big.tile([P, NBLK, NQ, NQ, KV], F16, tag="pr2")
            for b in range(NBLK):
                nc.vector.tensor_mul(
                    pr2[:, b],
                    q_t[:, b].unsqueeze(2).broadcast_to((P, NQ, NQ, KV)),
                    k_t[:, b].unsqueeze(1).broadcast_to((P, NQ, NQ, KV)),
                )
            t1sc = sm.tile([P, NBLK, NQ * NQ, 4], F16, tag="t1sc", bufs=1)
            nc.vector.tensor_add(
                t1sc[:],
                pr2[:].rearrange("p b i j d -> p b (i j) d")[:, :, :, 0:4],
                pr2[:].rearrange("p b i j d -> p b (i j) d")[:, :, :, 4:8],
            )
            t2sc = sm.tile([P, NBLK, NQ * NQ, 2], F16, tag="t2sc")
            nc.vector.tensor_add(t2sc[:], t1sc[:, :, :, 0:2], t1sc[:, :, :, 2:4])
            t3sc = sm.tile([P, NBLK, NQ * NQ], F16, tag="t3sc")
            sc = sm.tile([P, NBLK, NQ * NQ], F32, tag="sc")
            ex = sm.tile([P, NBLK, NQ, NQ], F16, tag="ex")
            pr2f = pr2[:].rearrange("p b i j d -> p b (i j) d")
            for b in range(NBLK):
                nc.vector.tensor_add(t3sc[:, b], t2sc[:, b, :, 0], t2sc[:, b, :, 1])
                nc.vector.tensor_add(sc[:, b], t3sc[:, b], pr2f[:, b, :, 8])
                # exp (ACT); shift keeps f16 range, absorbed by LN
                nc.scalar.activation(
                    ex[:, b].rearrange("p i j -> p (i j)"), sc[:, b],
                    AF.Exp, bias=shift_t, scale=inv_sqrt_kv,
                )

            if probe == 2:
                o_t2 = outp.tile([P, NBLK, D], F32, tag="o")
                nc.vector.tensor_copy(
                    o_t2[:].rearrange("p b (i d) -> p b i d", i=NQ)[:, :, :, 0:KV],
                    ex[:, :, :, 0:KV],
                )
                nc.sync.dma_start(
                    out=out_d[r0 : r0 + P * NBLK, :].rearrange(
                        "(b p) c -> p b c", b=NBLK
                    ),
                    in_=o_t2[:],
                )
                return

            # ---- attn @ v (+ Z via expm row): products + tree over j ----
            # pr3[p,b,i,e,j] = ex[p,b,i,j] * vE[p,b,e,j]; e=9 gives Z terms
            pr3 = big.tile([P, NBLK, NQ, KV + 1, NQ], F16, tag="pr3")
            for b in range(NBLK):
                nc.vector.tensor_mul(
                    pr3[:, b],
                    ex[:, b].unsqueeze(2).broadcast_to((P, NQ, KV + 1, NQ)),
                    vE[:, b].unsqueeze(1).broadcast_to((P, NQ, KV + 1, NQ)),
                )
            # av tree scratch aliases pr2's storage (dead after sc/r4 above)
            G = NQ * (KV + 1)  # 250 groups per block
            pr2flat = pr2[:].rearrange("p b i j d -> p b (i j d)")
            p3 = pr3[:].rearrange("p b i e j -> p b (i e) j")
            t1av = pr2flat[:, :, 0:G * 12].rearrange("p b (g w) -> p b g w", w=12)
            nc.vector.tensor_add(t1av, p3[:, :, :, 0:12], p3[:, :, :, 12:24])
            t2av = pr2flat[:, :, G * 12 : G * 18].rearrange(
                "p b (g w) -> p b g w", w=6
            )
            nc.vector.tensor_add(t2av, t1av[:, :, :, 0:6], t1av[:, :, :, 6:12])
            t3av_t = sm.tile([P, NBLK, G, 3], F16, tag="t3av")
            t3av = t3av_t[:]
            nc.vector.tensor_add(t3av, t2av[:, :, :, 0:3], t2av[:, :, :, 3:6])
            states.append(dict(t=t, r0=r0, xt=xt, t3av=t3av, p3=p3))


        def phase2(st):
            t, r0, xt, t3av, p3 = st["t"], st["r0"], st["xt"], st["t3av"], st["p3"]
            G = NQ * (KV + 1)
            s01 = sm.tile([P, NBLK, G], F16, tag="s01")
            nc.gpsimd.tensor_add(s01[:], t3av[:, :, :, 0], t3av[:, :, :, 1])
            s012 = sm.tile([P, NBLK, G], F16, tag="s012")
            nc.gpsimd.tensor_add(s012[:], s01[:], t3av[:, :, :, 2])
            u_t = outp.tile([P, NBLK, NQ, KV + 1], F32, tag="u")
            nc.gpsimd.tensor_add(
                u_t[:].rearrange("p b i e -> p b (i e)"), s012[:], p3[:, :, :, 24]
            )

            z_t = u_t[:, :, :, KV]  # [P, NBLK, NQ]

            if probe == 3:
                o_t3 = outp.tile([P, NBLK, D], F32, tag="o")
                nc.vector.tensor_mul(
                    o_t3[:].rearrange("p b (i d) -> p b i d", i=NQ),
                    xt[:].rearrange("p b (i d) -> p b i d", i=NQ),
                    z_t.unsqueeze(3).broadcast_to((P, NBLK, NQ, KV)),
                )
                nc.sync.dma_start(
                    out=out_d[r0 : r0 + P * NBLK, :].rearrange(
                        "(b p) c -> p b c", b=NBLK
                    ),
                    in_=o_t3[:],
                )
                return

            # ---- w = u + Z*x  (LN scale-invariance absorbs Z) ----
            # 3D views: [P, (b i), d] -- HW tensor ops allow <=3 free dims;
            # Pool engine only implements TensorTensor add/sub/mult (+copy).
            xv3 = xt[:].rearrange("p b (i d) -> p (b i) d", i=NQ)
            zb3 = z_t.rearrange("p b i -> p (b i)").unsqueeze(2).broadcast_to(
                (P, NBLK * NQ, KV)
            )
            zx = outp.tile([P, NBLK, NQ, KV], F32, tag="zx")
            zx3 = zx[:].rearrange("p b i d -> p (b i) d")
            nc.gpsimd.tensor_mul(zx3, xv3, zb3)
            w_t = outp.tile([P, NBLK, NQ, KV], F32, tag="w")
            w3 = w_t[:].rearrange("p b i d -> p (b i) d")
            nc.gpsimd.tensor_add(
                w3, u_t[:, :, :, 0:KV].rearrange("p b i d -> p (b i) d"), zx3
            )

            # wsq = (3w)^2 = 9w^2 (scale folded into ACT square)
            wsq = outp.tile([P, NBLK, NQ, KV], F32, tag="wsq")
            nc.scalar.activation(
                wsq[:].rearrange("p b i d -> p (b i d)"),
                w_t[:].rearrange("p b i d -> p (b i d)"),
                AF.Square, bias=0.0, scale=3.0,
            )
            # Pool 3+3+3 trees: sw = sum_e w ; sq9 = sum_e 9w^2
            sw = sm.tile([P, NBLK * NQ], F32, tag="sw")
            sq9 = sm.tile([P, NBLK * NQ], F32, tag="sq9")
            for srct, dst, tg in ((w_t, sw, "rw"), (wsq, sq9, "rq")):
                s3 = srct[:].rearrange("p b i d -> p (b i) d")
                r1 = sm.tile([P, NBLK * NQ, 3], F32, tag=f"{tg}1")
                nc.gpsimd.tensor_add(r1[:], s3[:, :, 0:3], s3[:, :, 3:6])
                r2 = sm.tile([P, NBLK * NQ, 3], F32, tag=f"{tg}2")
                nc.gpsimd.tensor_add(r2[:], r1[:], s3[:, :, 6:9])
                r3 = sm.tile([P, NBLK * NQ], F32, tag=f"{tg}3")
                nc.gpsimd.tensor_add(r3[:], r2[:, :, 0], r2[:, :, 1])
                nc.gpsimd.tensor_add(dst[:], r3[:], r2[:, :, 2])

            # var*81 = 9*sum(9w^2)/9... : v2 = sq9*... -- algebra:
            # sum 9w^2 = 9*sum w^2 ; var*81 = 9*(9*sum w^2) - (3*sw)^2... use:
            # 81*var = 9*sq9 - 9*sw^2 -> instead: v2 = sq9 - s2 where
            # s2 = sw^2 gives 9*(sum w^2) - sw^2 = 9*var*9 = 81*var/... :
            # sum w^2 = sq9/9; 81*var = 9*(9 sum w^2 - sw^2) = 9*(sq9 - sw^2)
            # so compute v2 = sq9 - s2, then fold the final *9 into z2/ln:
            # 81*(var + eps*Z^2) = 9*v2 + 81*eps*Z^2 = 9*(v2 + 9*eps*Z^2)
            s2 = sm.tile([P, NBLK * NQ], F32, tag="s2")
            nc.gpsimd.tensor_mul(s2[:], sw[:], sw[:])
            v2 = sm.tile([P, NBLK * NQ], F32, tag="v2")
            nc.gpsimd.tensor_sub(v2[:], sq9[:], s2[:])
            # z2 = (9*sqrt(eps)*Z)^2 = 81*eps*Z^2  (scale folded into square)
            z2 = sm.tile([P, NBLK * NQ], F32, tag="z2")
            nc.scalar.activation(
                z2[:], z_t.rearrange("p b i -> p (b i)"),
                AF.Square, bias=0.0, scale=float(9.0 * np.sqrt(EPS)),
            )
            v3 = sm.tile([P, NBLK * NQ], F32, tag="v3")
            nc.gpsimd.tensor_add(v3[:], z2[:], v2[:])
            # v3 = 81*(var + eps*Z^2) so exp(-ln(v3)/2) = rstd_true/9
            # -> gamma*9 on host
            lnv = sm.tile([P, NBLK * NQ], F32, tag="lnv")
            nc.scalar.activation(lnv[:], v3[:], AF.Ln, bias=0.0, scale=1.0)
            rstd = sm.tile([P, NBLK * NQ], F32, tag="rstd")
            nc.scalar.activation(rstd[:], lnv[:], AF.Exp, bias=0.0, scale=-0.5)

            # cen = w - sw/9  (the -1/9 scale via ACT copy)
            swm = sm.tile([P, NBLK * NQ], F32, tag="swm")
            nc.scalar.activation(
                swm[:], sw[:], AF.Copy, bias=0.0, scale=float(-1.0 / 9.0)
            )
            cen = outp.tile([P, NBLK, NQ, KV], F32, tag="cen")
            cen3 = cen[:].rearrange("p b i d -> p (b i) d")
            nc.gpsimd.tensor_add(
                cen3, w3, swm[:].unsqueeze(2).broadcast_to((P, NBLK * NQ, KV))
            )
            t4 = outp.tile([P, NBLK, NQ, KV], F32, tag="t4")
            t43 = t4[:].rearrange("p b i d -> p (b i) d")
            nc.gpsimd.tensor_mul(
                t43, cen3, rstd[:].unsqueeze(2).broadcast_to((P, NBLK * NQ, KV))
            )
            og = outp.tile([P, NBLK, NQ, KV], F32, tag="og")
            og3 = og[:].rearrange("p b i d -> p (b i) d")
            gb3 = g_t.unsqueeze(1).broadcast_to((P, NBLK * NQ, KV))
            bb3 = b_t.unsqueeze(1).broadcast_to((P, NBLK * NQ, KV))
            nc.gpsimd.tensor_mul(og3, t43, gb3)
            o_t = outp.tile([P, NBLK, NQ, KV], F32, tag="o")
            nc.gpsimd.tensor_add(o_t[:].rearrange("p b i d -> p (b i) d"), og3, bb3)
            nc.sync.dma_start(
                out=out_d[r0 : r0 + P * NBLK, :].rearrange("(b p) c -> p b c", b=NBLK),
                in_=o_t[:].rearrange("p b i d -> p b (i d)"),
            )


        for t in range(nsuper):
            phase1(t)
            if len(states) > 1:
                phase2(states.pop(0))
        while states:
            phase2(states.pop(0))

    _split_multi_waits(nc)
    return nc


def _split_multi_waits(nc):
    """Walrus allows only one sync-wait slot on most instruction encodings.
    Hoist excess waits into NoOps inserted just before the offender."""
    for f in nc.m.functions:
        for b in f.blocks:
            i = 0
            while i < len(b.instructions):
                inst = b.instructions[i]
                si = getattr(inst, "sync_info", None)
                if si is not None and si.on_wait and len(si.on_wait) > 1:
                    extra = si.on_wait[:-1]
                    si.on_wait = si.on_wait[-1:]
                    for w in extra:
                        nop = mybir.InstNoOp(
                            name=nc.get_next_instruction_name(),
                            engine=inst.engine,
                            ins=[],
                            outs=[],
                            sync_info=mybir.SyncInfo(on_wait=[w], on_update=[]),
                        )
                        nc.register_instruction(nop)
                        b.instructions.insert(i, nop)
                        i += 1
                i += 1
    return nc


_NC_CACHE = {}


def _get_program(b_core):
    if b_core not in _NC_CACHE:
        _NC_CACHE[b_core] = build_program(b_core)
    return _NC_CACHE[b_core]


def _host_consts(mask, Wq, bq, Wk, bk, Wv, bv, gamma, beta):
    """Build pm16 [128, PM16_COLS] f16 and cst [CST_LEN] f32."""
    gidx = np.empty(NQ, dtype=np.int64)
    for g, (s, e, n) in enumerate(GROUPS):
        gidx[s // KV : e // KV] = g

    mask = np.asarray(mask, np.float64)
    expm = np.exp(np.float64(-1e9) * mask).astype(np.float32)  # mask weights

    def mk_stationary(W, b_, tok_lo, tok_hi, scale_tok=None):
        n = (tok_hi - tok_lo) * KV
        M = np.zeros((n + 1, n), dtype=np.float32)
        for i in range(tok_lo, tok_hi):
            s = 1.0 if scale_tok is None else scale_tok[i]
            r = (i - tok_lo) * KV
            M[r : r + KV, r : r + KV] = np.asarray(W, np.float32)[gidx[i]].T * s
            M[n, r : r + KV] = np.asarray(b_, np.float32)[gidx[i]] * s
        return M

    pm16 = np.zeros((P, PM16_COLS), dtype=np.float16)
    pm16[:, 0:P] = np.eye(P, dtype=np.float16)
    o = P
    for W, b_, sc in ((Wq, bq, None), (Wk, bk, None), (Wv, bv, expm)):
        M = mk_stationary(W, b_, 0, 14, sc)
        pm16[0 : NA + 1, o : o + NA] = M.astype(np.float16)
        o += NA
    for W, b_, sc in ((Wq, bq, None), (Wk, bk, None), (Wv, bv, expm)):
        M = mk_stationary(W, b_, 14, 25, sc)
        pm16[0 : NB_ + 1, o : o + NB_] = M.astype(np.float16)
        o += NB_
    assert o == PM16_COLS

    cst = np.concatenate([
        9.0 * np.asarray(gamma, np.float32).reshape(-1),  # rstd is rstd_true/9
        np.asarray(beta, np.float32).reshape(-1),
        expm.reshape(-1),
        np.array([SHIFT], np.float32),
    ]).astype(np.float32)
    assert cst.shape[0] == CST_LEN
    id32 = np.eye(P, dtype=np.float32)
    return pm16, cst, id32


def kernel(x, mask, Wq, bq, Wk, bk, Wv, bv, gamma, beta):
    x = np.ascontiguousarray(np.asarray(x, dtype=np.float32))
    B = x.shape[0]
    b_core = B // N_CORES
    pm16, cst, id32 = _host_consts(mask, Wq, bq, Wk, bk, Wv, bv, gamma, beta)

    nc = _get_program(b_core)
    shards = x.reshape(N_CORES, b_core, D)
    in_maps = []
    for c in range(N_CORES):
        in_maps.append({
            "x": np.ascontiguousarray(shards[c]),
            "cst": cst,
            "id32": id32,
            "pm16": pm16,
            "ones16": np.ones((NBLK * P,), dtype=np.float16),
        })
    res = run_bass_kernel_spmd(nc, in_maps, core_ids=list(range(N_CORES)))
    outs = [res.results[c]["out"] for c in range(N_CORES)]
    full = np.concatenate(outs, axis=0).reshape(B, NQ, KV)
    return full.astype(np.float32)
